# revision 1
# baseline (speedup 1.0000x reference)
"""Paged GQA decode attention (sparse_attention) on 8 TRN2 NeuronCores.

Sharding: tensor-parallel by KV head (8 heads -> 8 cores). Each core gets its
head's slice of the KV pool, pre-split on host into interleaved bf16 hi|lo
rows so that:
  - dma_gather(transpose=True) delivers K^T tiles (d on partitions) directly,
    sidestepping the f32 on-chip transpose problem, at full f32 fidelity
    (hi+lo bf16 pair) and identical DMA bytes (2x256B = 512B/token).
  - V gathers land in natural [s,d] layout for the PV matmul.

Per core dataflow (fully specialized at build time on the actual seq_lens /
pool-half split, which is identical across cores):
  QK:   scores^T[s,4] = khi^T@qhi + khi^T@qlo + klo^T@qhi   (PSUM, batched
        128 slot-columns per bank)
  exp:  one ACT Exp per group bank -> p^T in SBUF (f32), pad tails masked by
        per-section mask columns (tensor_scalar per-partition multiply)
  PV:   o^T accum: phi@vhi + plo@vhi + phi@vlo  (p split hi/lo bf16)
  sums: ones-vector matmul -> per-slot partial sums; final reduction and
        softmax normalization happen on host (division is elementwise on the
        tiny [B,HQ,D] output).
"""

import os

import numpy as np
import ml_dtypes

import concourse.bacc as bacc
import concourse.bass as bass
import concourse.mybir as mybir
import concourse.tile as tile
from concourse.bass_utils import run_bass_kernel_spmd

B, S, HQ, HKV, D, G = 32, 2048, 32, 8, 128, 4
POOL = B * S
HALF = POOL // 2
SCALE = D ** -0.5
NCORES = 8
GROUPS = 16
RPG = B // GROUPS  # requests per group

BF16 = ml_dtypes.bfloat16

_prog_cache: dict = {}
LAST_RESULT = None  # test.py introspection (exec time etc.)


def _pad128(n):
    return (n + 127) // 128 * 128


def _layout(meta):
    """meta[g][h][j] = valid token count of request j in half h of group g.

    Returns bookkeeping: per group: n_gh (padded positions per half),
    slot tables, per-request slot lists + sum ranges + mask column ids,
    plus each group-half's column offset into the merged idx tensor.
    """
    info = []
    mask_cols = []  # list of (g, h, j, valid_in_last_slot) -> mask col id
    icol = 0  # running column offset into the merged idx tensor
    for g in range(GROUPS):
        lo_secs, hi_secs = meta[g]
        halves = []
        for h, secs in enumerate((lo_secs, hi_secs)):
            starts, slot_cnt = [], []
            pos = 0
            for j in range(RPG):
                starts.append(pos // 128)
                slot_cnt.append(_pad128(secs[j]) // 128)
                pos += _pad128(secs[j])
            halves.append(dict(n=pos, slots=pos // 128, ioff=icol,
                               starts=starts, slot_cnt=slot_cnt, secs=secs))
            icol += pos // 16
        n_lo_slots = halves[0]["slots"]
        nslots = n_lo_slots + halves[1]["slots"]
        # per request: list of (half, local_slot, global_slot)
        req_slots, req_ranges, req_masks = [], [], []
        for j in range(RPG):
            slots, ranges, masks = [], [], []
            for h in (0, 1):
                hh = halves[h]
                base = 0 if h == 0 else n_lo_slots
                s0, cnt = hh["starts"][j], hh["slot_cnt"][j]
                if cnt:
                    ranges.append((base + s0, cnt))
                    for li in range(cnt):
                        slots.append((h, s0 + li, base + s0 + li))
                    tail = hh["secs"][j] % 128
                    if tail:  # partial last slot -> needs mask col
                        mid = len(mask_cols)
                        mask_cols.append((g, h, j, tail))
                        masks.append((base + s0 + cnt - 1, mid))
            req_slots.append(slots)
            req_ranges.append(ranges)
            req_masks.append(masks)
        info.append(dict(halves=halves, nslots=nslots,
                         req_slots=req_slots, req_ranges=req_ranges,
                         req_masks=req_masks))
    return info, mask_cols, icol


def _build_program(meta):
    info, mask_cols, idx_w = _layout(meta)
    n_mask = max(1, len(mask_cols))
    dt = mybir.dt
    nc = bacc.Bacc(trn_type="TRN2")

    k_il = nc.dram_tensor("k_il", [POOL, 256], dt.bfloat16, kind="ExternalInput")
    v_il = nc.dram_tensor("v_il", [POOL, 256], dt.bfloat16, kind="ExternalInput")
    qhiT = nc.dram_tensor("qhiT", [128, 128], dt.bfloat16, kind="ExternalInput")
    qloT = nc.dram_tensor("qloT", [128, 128], dt.bfloat16, kind="ExternalInput")
    maskc_d = nc.dram_tensor("maskc", [128, n_mask], dt.float32, kind="ExternalInput")
    idx_w = max(1, idx_w)
    idx_d = nc.dram_tensor("idx_all", [128, idx_w], dt.int16, kind="ExternalInput")
    OC = RPG * D  # output cols per group
    o_dram = nc.dram_tensor("o_un", [G, B * D], dt.float32, kind="ExternalOutput")
    s_dram = nc.dram_tensor("sums", [GROUPS, 512], dt.float32, kind="ExternalOutput")

    with tile.TileContext(nc) as tc:
        with (
            tc.tile_pool(name="const", bufs=1) as cpool,
            tc.tile_pool(name="kt", bufs=4) as ktp,
            tc.tile_pool(name="vt", bufs=4) as vtp,
            tc.tile_pool(name="pt", bufs=2) as ptp,
            tc.tile_pool(name="stg", bufs=2) as stgp,
            tc.tile_pool(name="ps_sc", bufs=2, space="PSUM") as pssc,
            tc.tile_pool(name="ps_pv", bufs=2, space="PSUM") as pspv,
            tc.tile_pool(name="ps_sm", bufs=2, space="PSUM") as pssm,
        ):
            qhi_t = cpool.tile([128, 128], dt.bfloat16, tag="qhi")
            qlo_t = cpool.tile([128, 128], dt.bfloat16, tag="qlo")
            ones_t = cpool.tile([128, 1], dt.float32, tag="ones")
            mask_t = cpool.tile([128, n_mask], dt.float32, tag="maskc")
            idx_t = cpool.tile([128, idx_w], dt.int16, tag="idxall")
            _w0 = info[1]["halves"][1]["ioff"] if GROUPS > 1 else idx_w
            _w0 = max(1, min(_w0, idx_w))
            nc.sync.dma_start(out=idx_t[:, 0:_w0], in_=idx_d[:, 0:_w0])
            if _w0 < idx_w:
                nc.sync.dma_start(out=idx_t[:, _w0:idx_w], in_=idx_d[:, _w0:idx_w])
            nc.sync.dma_start(out=qhi_t[:], in_=qhiT[:])
            nc.sync.dma_start(out=qlo_t[:], in_=qloT[:])
            nc.sync.dma_start(out=mask_t[:], in_=maskc_d[:])
            nc.vector.memset(ones_t[:], 1.0)

            for g in range(GROUPS):
                gi = info[g]
                nslots = gi["nslots"]
                ncols = 4 * nslots
                if nslots == 0:
                    # all requests in this group are empty (degenerate input)
                    z = stgp.tile([G, OC], dt.float32, tag="ostg")
                    nc.vector.memset(z[:], 0.0)
                    nc.sync.dma_start(out=o_dram[0:G, OC * g:OC * (g + 1)],
                                      in_=z[:])
                    continue
                # --- gather K^T and V for both pool halves -----------------
                kt_tiles, v_tiles = {}, {}
                for h in (0, 1):
                    n = gi["halves"][h]["n"]
                    if n == 0:
                        continue
                    ioff = gi["halves"][h]["ioff"]
                    it = idx_t[:, ioff:ioff + n // 16]
                    src_k = k_il[0:HALF, :] if h == 0 else k_il[HALF:POOL, :]
                    src_v = v_il[0:HALF, :] if h == 0 else v_il[HALF:POOL, :]
                    kt = ktp.tile([128, 2, n], dt.bfloat16, tag="kt")
                    nc.gpsimd.dma_gather(
                        out_ap=kt[:], in_ap=src_k, idxs_ap=it,
                        num_idxs=n, num_idxs_reg=n, elem_size=256,
                        transpose=True, single_packet=False)
                    vt = vtp.tile([128, n // 128, 256], dt.bfloat16, tag="vt")
                    nc.gpsimd.dma_gather(
                        out_ap=vt[:], in_ap=src_v, idxs_ap=it,
                        num_idxs=n, num_idxs_reg=n, elem_size=256,
                        transpose=False, single_packet=False)
                    kt_tiles[h] = kt
                    v_tiles[h] = vt

                # --- QK: scores^T into one PSUM bank -----------------------
                sc = pssc.tile([128, ncols], dt.float32, tag="sc")
                n_lo_slots = gi["halves"][0]["slots"]
                for s in range(nslots):
                    h = 0 if s < n_lo_slots else 1
                    loc = s if h == 0 else s - n_lo_slots
                    # owner request of this slot
                    hh = gi["halves"][h]
                    j = max(jj for jj in range(RPG) if hh["starts"][jj] <= loc)
                    b = RPG * g + j
                    kt = kt_tiles[h]
                    khiT = kt[:, 0, 128 * loc:128 * (loc + 1)]
                    kloT = kt[:, 1, 128 * loc:128 * (loc + 1)]
                    out = sc[:, 4 * s:4 * s + 4]
                    qh = qhi_t[:, 4 * b:4 * b + 4]
                    ql = qlo_t[:, 4 * b:4 * b + 4]
                    nc.tensor.matmul(out, khiT, qh, start=True, stop=False)
                    nc.tensor.matmul(out, khiT, ql, start=False, stop=False)
                    nc.tensor.matmul(out, kloT, qh, start=False, stop=True)

                # --- softmax numerator (no max-subtraction; scores are O(1))
                pt = ptp.tile([128, ncols], dt.float32, tag="pt")
                nc.scalar.activation(pt[:], sc[:],
                                     mybir.ActivationFunctionType.Exp)
                # zero the padded tail positions of each section
                for j in range(RPG):
                    for (gslot, mid) in gi["req_masks"][j]:
                        cc = 4 * gslot
                        nc.vector.tensor_scalar_mul(
                            out=pt[:, cc:cc + 4], in0=pt[:, cc:cc + 4],
                            scalar1=mask_t[:, mid:mid + 1])
                phi = ptp.tile([128, ncols], dt.bfloat16, tag="phi")
                plo = ptp.tile([128, ncols], dt.bfloat16, tag="plo")
                nc.vector.tensor_copy(out=phi[:], in_=pt[:])
                nc.vector.tensor_tensor(out=plo[:], in0=pt[:], in1=phi[:],
                                        op=mybir.AluOpType.subtract)

                # --- PV + sums --------------------------------------------
                pv = pspv.tile([G, OC], dt.float32, tag="pv")
                sm = pssm.tile([1, 512], dt.float32, tag="sm")
                for j in range(RPG):
                    slots = gi["req_slots"][j]
                    oc = 128 * j
                    if not slots:
                        nc.vector.memset(pv[0:G, oc:oc + 128], 0.0)
                        continue
                    last = len(slots) - 1
                    for si, (h, loc, gslot) in enumerate(slots):
                        vt = v_tiles[h]
                        vhi = vt[:, loc, 0:128]
                        vlo = vt[:, loc, 128:256]
                        ph = phi[:, 4 * gslot:4 * gslot + 4]
                        pl = plo[:, 4 * gslot:4 * gslot + 4]
                        out = pv[0:G, oc:oc + 128]
                        nc.tensor.matmul(out, ph, vhi,
                                         start=(si == 0), stop=False)
                        nc.tensor.matmul(out, pl, vhi, start=False, stop=False)
                        nc.tensor.matmul(out, ph, vlo, start=False,
                                         stop=(si == last))
                    for (s0, cnt) in gi["req_ranges"][j]:
                        nc.tensor.matmul(
                            sm[0:1, 4 * s0:4 * (s0 + cnt)], ones_t[:, 0:1],
                            pt[:, 4 * s0:4 * (s0 + cnt)], start=True, stop=True)

                ostg = stgp.tile([G, OC], dt.float32, tag="ostg")
                sstg = stgp.tile([1, 512], dt.float32, tag="sstg")
                nc.vector.tensor_copy(out=ostg[:], in_=pv[:])
                nc.vector.tensor_copy(out=sstg[0:1, 0:ncols],
                                      in_=sm[0:1, 0:ncols])
                nc.sync.dma_start(out=o_dram[0:G, OC * g:OC * (g + 1)],
                                  in_=ostg[:])
                nc.sync.dma_start(out=s_dram[g:g + 1, 0:ncols],
                                  in_=sstg[0:1, 0:ncols])

    nc.compile()
    return nc, info, mask_cols


def prepare(inputs):
    q = np.asarray(inputs["q"], np.float32)
    k = np.asarray(inputs["k"], np.float32)
    v = np.asarray(inputs["v"], np.float32)
    k_buffer = np.asarray(inputs["k_buffer"], np.float32)
    v_buffer = np.asarray(inputs["v_buffer"], np.float32)
    req_to_token = np.asarray(inputs["req_to_token"])
    req_pool_indices = np.asarray(inputs["req_pool_indices"])
    seq_lens = np.asarray(inputs["seq_lens"]).astype(np.int64)
    out_cache_loc = np.asarray(inputs["out_cache_loc"]).astype(np.int64)

    # store_kv_cache scatter (tiny: 32 rows) + per-request token lists
    kb = k_buffer.copy()
    vb = v_buffer.copy()
    kb[out_cache_loc] = k.reshape(B, HKV, D)
    vb[out_cache_loc] = v.reshape(B, HKV, D)
    tok = req_to_token[req_pool_indices]

    # smallest group first (fast pipeline fill), next-smallest last (short
    # drain tail), the rest biggest-first in between
    asc = list(np.argsort(seq_lens, kind="stable"))
    head, tail_, mid = asc[:RPG], asc[RPG:2 * RPG], asc[2 * RPG:][::-1]
    order = np.array(head + mid + tail_, dtype=np.int64)

    meta = []
    idx_blocks = []
    for g in range(GROUPS):
        lo_secs, hi_secs = [], []
        for h in (0, 1):
            parts = []
            secs = lo_secs if h == 0 else hi_secs
            for j in range(RPG):
                b = int(order[RPG * g + j])
                t = tok[b, :seq_lens[b]].astype(np.int64)
                tl = t[t < HALF] if h == 0 else t[t >= HALF] - HALF
                secs.append(len(tl))
                arr = np.zeros(_pad128(len(tl)), np.int64)
                arr[:len(tl)] = tl
                parts.append(arr)
            full = np.concatenate(parts)
            if len(full):
                # [16, n/16] wrap, replicated into all 8 GPSIMD-core stripes
                idx_blocks.append(
                    np.tile(full.astype(np.int16).reshape(-1, 16).T, (8, 1)))
        meta.append((tuple(lo_secs), tuple(hi_secs)))
    meta = tuple(meta)
    if idx_blocks:
        idx_all = np.ascontiguousarray(np.concatenate(idx_blocks, axis=1))
    else:
        idx_all = np.zeros((128, 1), np.int16)

    if meta not in _prog_cache:
        _prog_cache[meta] = _build_program(meta)
    nc, info, mask_cols = _prog_cache[meta]

    maskc = np.ones((128, max(1, len(mask_cols))), np.float32)
    for mid, (_, _, _, tail) in enumerate(mask_cols):
        maskc[:, mid] = (np.arange(128) < tail).astype(np.float32)

    in_maps = []
    for c in range(NCORES):
        kh = kb[:, c, :]
        vh = vb[:, c, :]
        k_hi = kh.astype(BF16)
        k_lo = (kh - k_hi.astype(np.float32)).astype(BF16)
        v_hi = vh.astype(BF16)
        v_lo = (vh - v_hi.astype(np.float32)).astype(BF16)
        qc = (q.reshape(B, HKV, G, D)[order, c] * SCALE).reshape(B * G, D)
        qT = np.ascontiguousarray(qc.T)
        q_hi = qT.astype(BF16)
        q_lo = (qT - q_hi.astype(np.float32)).astype(BF16)
        im = {
            "k_il": np.ascontiguousarray(np.concatenate([k_hi, k_lo], axis=1)),
            "v_il": np.ascontiguousarray(np.concatenate([v_hi, v_lo], axis=1)),
            "qhiT": np.ascontiguousarray(q_hi),
            "qloT": np.ascontiguousarray(q_lo),
            "maskc": maskc,
            "idx_all": idx_all,
        }
        in_maps.append(im)
    return nc, info, in_maps, order


def postprocess(results, info, order, cores=None):
    OC = RPG * D
    out = np.zeros((B, HQ, D), np.float32)
    for c in (cores if cores is not None else range(NCORES)):
        o_un = results[c]["o_un"]
        sums = results[c]["sums"]
        for g in range(GROUPS):
            gi = info[g]
            for j in range(RPG):
                b = int(order[RPG * g + j])
                stot = np.zeros(G, np.float64)
                for (s0, cnt) in gi["req_ranges"][j]:
                    seg = sums[g, 4 * s0:4 * (s0 + cnt)].astype(np.float64)
                    stot += seg.reshape(cnt, G).sum(axis=0)
                ov = o_un[:, OC * g + 128 * j:OC * g + 128 * (j + 1)]
                with np.errstate(divide="ignore", invalid="ignore"):
                    out[b, c * G:(c + 1) * G, :] = ov / stot[:, None]
    return out.reshape(B, HQ * D).astype(np.float32)


def kernel(**inputs):
    global LAST_RESULT
    nc, info, in_maps, order = prepare(inputs)
    res = run_bass_kernel_spmd(nc, in_maps, core_ids=list(range(NCORES)),
                               trace=False)
    LAST_RESULT = res
    return postprocess(res.results, info, order)



# revision 2
# speedup vs baseline: 1.7801x; 1.7801x over previous
"""Paged GQA decode attention (sparse_attention) on 8 TRN2 NeuronCores.

Sharding: tensor-parallel by KV head (8 heads -> 8 cores). Each core gets its
head's slice of the KV pool, pre-merged on host into single bf16 rows
[khi(128) | vhi(128)] (512 B) so that ONE natural dma_gather per (group,
pool-half) fetches both K and V for a token: half the HBM bytes and half the
descriptors of the previous hi/lo scheme, at the 512 B descriptor size that
avoids the sub-512B DMA latency penalty.

Per core dataflow (fully specialized at build time on the actual seq_lens /
pool-half split, identical across cores):
  gather: kv[tok, 0:128]=K, kv[tok, 128:256]=V  (natural layout, tok on
          partitions, one 512 B descriptor per token)
  K^T:    per 128-token slot, PE transpose K chunk -> PSUM (bf16), batched
          8 slots/bank; PSUM->SBUF copies alternate between DVE and ACT
  QK:     scores^T[tok,4] = ktT @ (qhi|qlo)   (2 matmuls, 4-col streams)
  exp:    one ACT Exp per group bank -> p^T in SBUF (f32); pad tails masked
          by per-section mask columns (tensor_scalar per-partition multiply)
  PV:     o^T[d,4] accum with V-natural stationary: phi,plo 4-col streams
  sums:   ones-vector matmul over phi+plo -> per-slot partial sums; final
          reduction and softmax normalization happen on host (elementwise on
          the tiny [B,HQ,D] output).
All matmuls stream only 4-column moving operands; the 128-wide data chunks
ride in the free stationary (Ldweights) or transpose paths.
"""

import os

import numpy as np
import ml_dtypes

import concourse.bacc as bacc
import concourse.bass as bass
import concourse.mybir as mybir
import concourse.tile as tile
from concourse.bass_utils import run_bass_kernel_spmd

B, S, HQ, HKV, D = 32, 2048, 32, 8, 128
G = HQ // HKV
POOL = B * S
HALF = POOL // 2
SCALE = D ** -0.5
NCORES = 8
GROUPS = 8
RPG = B // GROUPS  # requests per group
TB = 8             # K^T transpose slots per PSUM bank / copy batch

BF16 = ml_dtypes.bfloat16

_prog_cache: dict = {}
LAST_RESULT = None  # test.py introspection (exec time etc.)


def _pad128(n):
    return (n + 127) // 128 * 128


def _layout(meta):
    """meta[g][h][j] = valid token count of request j in half h of group g.

    Returns bookkeeping: per group: merged slot table (lo-half slots then
    hi-half slots), per-request slot lists + sum ranges + mask column ids,
    plus each group-half's column offset into the merged idx tensor.
    """
    info = []
    mask_cols = []  # list of (g, h, j, valid_in_last_slot) -> mask col id
    icol = 0  # running column offset into the merged idx tensor
    for g in range(GROUPS):
        lo_secs, hi_secs = meta[g]
        halves = []
        for h, secs in enumerate((lo_secs, hi_secs)):
            starts, slot_cnt = [], []
            pos = 0
            for j in range(RPG):
                starts.append(pos // 128)
                slot_cnt.append(_pad128(secs[j]) // 128)
                pos += _pad128(secs[j])
            halves.append(dict(n=pos, slots=pos // 128, ioff=icol,
                               starts=starts, slot_cnt=slot_cnt, secs=secs))
            icol += pos // 16
        n_lo_slots = halves[0]["slots"]
        nslots = n_lo_slots + halves[1]["slots"]
        # per request: global slot ids, contiguous ranges, masked tail slots
        req_slots, req_ranges, req_masks = [], [], []
        for j in range(RPG):
            slots, ranges, masks = [], [], []
            for h in (0, 1):
                hh = halves[h]
                base = 0 if h == 0 else n_lo_slots
                s0, cnt = hh["starts"][j], hh["slot_cnt"][j]
                if cnt:
                    ranges.append((base + s0, cnt))
                    slots.extend(range(base + s0, base + s0 + cnt))
                    tail = hh["secs"][j] % 128
                    if tail:  # partial last slot -> needs mask col
                        mid = len(mask_cols)
                        mask_cols.append((g, h, j, tail))
                        masks.append((base + s0 + cnt - 1, mid))
            req_slots.append(slots)
            req_ranges.append(ranges)
            req_masks.append(masks)
        info.append(dict(halves=halves, nslots=nslots,
                         req_slots=req_slots, req_ranges=req_ranges,
                         req_masks=req_masks))
    return info, mask_cols, icol


def _build_program(meta):
    info, mask_cols, idx_w = _layout(meta)
    n_mask = max(1, len(mask_cols))
    dt = mybir.dt
    nc = bacc.Bacc(trn_type="TRN2")

    kv_il = nc.dram_tensor("kv_il", [POOL, 256], dt.bfloat16, kind="ExternalInput")
    qhiT = nc.dram_tensor("qhiT", [128, 128], dt.bfloat16, kind="ExternalInput")
    qloT = nc.dram_tensor("qloT", [128, 128], dt.bfloat16, kind="ExternalInput")
    identd = nc.dram_tensor("identd", [128, 128], dt.bfloat16, kind="ExternalInput")
    maskc_d = nc.dram_tensor("maskc", [128, n_mask], dt.float32, kind="ExternalInput")
    idx_w = max(1, idx_w)
    idx_d = nc.dram_tensor("idx_all", [128, idx_w], dt.int16, kind="ExternalInput")
    OC = RPG * G  # output cols per group (o^T: one col per (req, q-head))
    o_dram = nc.dram_tensor("o_un", [128, B * G], dt.float32, kind="ExternalOutput")
    s_dram = nc.dram_tensor("sums", [GROUPS, 512], dt.float32, kind="ExternalOutput")

    with tile.TileContext(nc) as tc:
        with (
            tc.tile_pool(name="const", bufs=1) as cpool,
            tc.tile_pool(name="kv", bufs=2) as kvp,
            tc.tile_pool(name="ktT", bufs=2) as ktp,
            tc.tile_pool(name="pt", bufs=2) as ptp,
            tc.tile_pool(name="stg", bufs=2) as stgp,
            tc.tile_pool(name="ps_kt", bufs=2, space="PSUM") as pskt,
            tc.tile_pool(name="ps_sc", bufs=2, space="PSUM") as pssc,
            tc.tile_pool(name="ps_pv", bufs=2, space="PSUM") as pspv,
            tc.tile_pool(name="ps_sm", bufs=2, space="PSUM") as pssm,
        ):
            qhi_t = cpool.tile([128, 128], dt.bfloat16, tag="qhi")
            qlo_t = cpool.tile([128, 128], dt.bfloat16, tag="qlo")
            ident_t = cpool.tile([128, 128], dt.bfloat16, tag="ident")
            ones_t = cpool.tile([128, 1], dt.bfloat16, tag="ones")
            mask_t = cpool.tile([128, n_mask], dt.float32, tag="maskc")
            idx_t = cpool.tile([128, idx_w], dt.int16, tag="idxall")
            _w0 = info[1]["halves"][1]["ioff"] if GROUPS > 1 else idx_w
            _w0 = max(1, min(_w0, idx_w))
            nc.sync.dma_start(out=idx_t[:, 0:_w0], in_=idx_d[:, 0:_w0])
            if _w0 < idx_w:
                nc.sync.dma_start(out=idx_t[:, _w0:idx_w], in_=idx_d[:, _w0:idx_w])
            nc.sync.dma_start(out=qhi_t[:], in_=qhiT[:])
            nc.sync.dma_start(out=qlo_t[:], in_=qloT[:])
            nc.sync.dma_start(out=ident_t[:], in_=identd[:])
            nc.sync.dma_start(out=mask_t[:], in_=maskc_d[:])
            nc.vector.memset(ones_t[:], 1.0)

            copy_alt = 0  # alternate K^T copies between DVE and ACT
            for g in range(GROUPS):
                gi = info[g]
                nslots = gi["nslots"]
                ncols = 4 * nslots
                if nslots == 0:
                    # all requests in this group are empty (degenerate input)
                    z = stgp.tile([128, OC], dt.float32, tag="ostg")
                    nc.vector.memset(z[:], 0.0)
                    nc.sync.dma_start(out=o_dram[:, OC * g:OC * (g + 1)],
                                      in_=z[:])
                    continue
                # --- one merged K|V gather per pool half ------------------
                kvt = kvp.tile([128, nslots, 256], dt.bfloat16, tag="kv")
                n_lo_slots = gi["halves"][0]["slots"]
                for h in (0, 1):
                    n = gi["halves"][h]["n"]
                    if n == 0:
                        continue
                    ioff = gi["halves"][h]["ioff"]
                    it = idx_t[:, ioff:ioff + n // 16]
                    src = kv_il[0:HALF, :] if h == 0 else kv_il[HALF:POOL, :]
                    sb = 0 if h == 0 else n_lo_slots
                    nc.gpsimd.dma_gather(
                        out_ap=kvt[:, sb:sb + n // 128, :], in_ap=src,
                        idxs_ap=it, num_idxs=n, num_idxs_reg=n, elem_size=256,
                        transpose=False, single_packet=False)

                # --- K^T: PE transpose batches + PSUM->SBUF copies --------
                ktT = ktp.tile([128, nslots * 128], dt.bfloat16, tag="ktT")
                for s0 in range(0, nslots, TB):
                    nb = min(TB, nslots - s0)
                    kt_ps = pskt.tile([128, TB * 128], dt.bfloat16, tag="ktps")
                    for i in range(nb):
                        nc.tensor.transpose(kt_ps[:, 128 * i:128 * (i + 1)],
                                            kvt[:, s0 + i, 0:128], ident_t[:])
                    dst = ktT[:, 128 * s0:128 * (s0 + nb)]
                    if copy_alt % 3 == 2:
                        nc.scalar.activation(dst, kt_ps[:, 0:128 * nb],
                                             mybir.ActivationFunctionType.Copy)
                    else:
                        nc.vector.tensor_copy(out=dst, in_=kt_ps[:, 0:128 * nb])
                    copy_alt += 1

                # --- QK: scores^T into one PSUM bank ----------------------
                sc = pssc.tile([128, ncols], dt.float32, tag="sc")
                for s in range(nslots):
                    h = 0 if s < n_lo_slots else 1
                    loc = s if h == 0 else s - n_lo_slots
                    hh = gi["halves"][h]
                    j = max(jj for jj in range(RPG) if hh["starts"][jj] <= loc)
                    b = RPG * g + j
                    kT = ktT[:, 128 * s:128 * (s + 1)]
                    out = sc[:, 4 * s:4 * s + 4]
                    nc.tensor.matmul(out, kT, qhi_t[:, 4 * b:4 * b + 4],
                                     start=True, stop=False)
                    nc.tensor.matmul(out, kT, qlo_t[:, 4 * b:4 * b + 4],
                                     start=False, stop=True)

                # --- softmax numerator (no max-subtraction; scores are O(1))
                pt = ptp.tile([128, ncols], dt.float32, tag="pt")
                nc.scalar.activation(pt[:], sc[:],
                                     mybir.ActivationFunctionType.Exp)
                # zero the padded tail positions of each section
                for j in range(RPG):
                    for (gslot, mid) in gi["req_masks"][j]:
                        cc = 4 * gslot
                        nc.vector.tensor_scalar_mul(
                            out=pt[:, cc:cc + 4], in0=pt[:, cc:cc + 4],
                            scalar1=mask_t[:, mid:mid + 1])
                phi = ptp.tile([128, ncols], dt.bfloat16, tag="phi")
                plo = ptp.tile([128, ncols], dt.bfloat16, tag="plo")
                nc.vector.tensor_copy(out=phi[:], in_=pt[:])
                nc.vector.tensor_tensor(out=plo[:], in0=pt[:], in1=phi[:],
                                        op=mybir.AluOpType.subtract)

                # --- PV (o^T accum, V-natural stationary) + sums ----------
                pv = pspv.tile([128, OC], dt.float32, tag="pv")
                sm = pssm.tile([1, 512], dt.float32, tag="sm")
                for j in range(RPG):
                    slots = gi["req_slots"][j]
                    oc = G * j
                    if not slots:
                        nc.vector.memset(pv[:, oc:oc + G], 0.0)
                        continue
                    last = len(slots) - 1
                    for si, s in enumerate(slots):
                        vsl = kvt[:, s, 128:256]
                        out = pv[:, oc:oc + G]
                        nc.tensor.matmul(out, vsl, phi[:, 4 * s:4 * s + 4],
                                         start=(si == 0), stop=False)
                        nc.tensor.matmul(out, vsl, plo[:, 4 * s:4 * s + 4],
                                         start=False, stop=(si == last))
                    for (s0, cnt) in gi["req_ranges"][j]:
                        so = sm[0:1, 4 * s0:4 * (s0 + cnt)]
                        nc.tensor.matmul(so, ones_t[:, 0:1],
                                         phi[:, 4 * s0:4 * (s0 + cnt)],
                                         start=True, stop=False)
                        nc.tensor.matmul(so, ones_t[:, 0:1],
                                         plo[:, 4 * s0:4 * (s0 + cnt)],
                                         start=False, stop=True)

                ostg = stgp.tile([128, OC], dt.float32, tag="ostg")
                sstg = stgp.tile([1, 512], dt.float32, tag="sstg")
                nc.vector.tensor_copy(out=ostg[:], in_=pv[:])
                nc.vector.tensor_copy(out=sstg[0:1, 0:ncols],
                                      in_=sm[0:1, 0:ncols])
                nc.sync.dma_start(out=o_dram[:, OC * g:OC * (g + 1)],
                                  in_=ostg[:])
                nc.sync.dma_start(out=s_dram[g:g + 1, 0:ncols],
                                  in_=sstg[0:1, 0:ncols])

    nc.compile()
    return nc, info, mask_cols


def prepare(inputs):
    q = np.asarray(inputs["q"], np.float32)
    k = np.asarray(inputs["k"], np.float32)
    v = np.asarray(inputs["v"], np.float32)
    k_buffer = np.asarray(inputs["k_buffer"], np.float32)
    v_buffer = np.asarray(inputs["v_buffer"], np.float32)
    req_to_token = np.asarray(inputs["req_to_token"])
    req_pool_indices = np.asarray(inputs["req_pool_indices"])
    seq_lens = np.asarray(inputs["seq_lens"]).astype(np.int64)
    out_cache_loc = np.asarray(inputs["out_cache_loc"]).astype(np.int64)

    # store_kv_cache scatter (tiny: 32 rows) + per-request token lists
    kb = k_buffer.copy()
    vb = v_buffer.copy()
    kb[out_cache_loc] = k.reshape(B, HKV, D)
    vb[out_cache_loc] = v.reshape(B, HKV, D)
    tok = req_to_token[req_pool_indices]

    # smallest group first (fast pipeline fill), next-smallest last (short
    # drain tail), the rest biggest-first in between
    asc = list(np.argsort(seq_lens, kind="stable"))
    head, tail_, mid = asc[:RPG], asc[RPG:2 * RPG], asc[2 * RPG:][::-1]
    order = np.array(head + mid + tail_, dtype=np.int64)

    meta = []
    idx_blocks = []
    for g in range(GROUPS):
        lo_secs, hi_secs = [], []
        for h in (0, 1):
            parts = []
            secs = lo_secs if h == 0 else hi_secs
            for j in range(RPG):
                b = int(order[RPG * g + j])
                t = tok[b, :seq_lens[b]].astype(np.int64)
                tl = t[t < HALF] if h == 0 else t[t >= HALF] - HALF
                secs.append(len(tl))
                arr = np.zeros(_pad128(len(tl)), np.int64)
                arr[:len(tl)] = tl
                parts.append(arr)
            full = np.concatenate(parts)
            if len(full):
                # [16, n/16] wrap, replicated into all 8 GPSIMD-core stripes
                idx_blocks.append(
                    np.tile(full.astype(np.int16).reshape(-1, 16).T, (8, 1)))
        meta.append((tuple(lo_secs), tuple(hi_secs)))
    meta = tuple(meta)
    if idx_blocks:
        idx_all = np.ascontiguousarray(np.concatenate(idx_blocks, axis=1))
    else:
        idx_all = np.zeros((128, 1), np.int16)

    if meta not in _prog_cache:
        _prog_cache[meta] = _build_program(meta)
    nc, info, mask_cols = _prog_cache[meta]

    maskc = np.ones((128, max(1, len(mask_cols))), np.float32)
    for mid, (_, _, _, tail) in enumerate(mask_cols):
        maskc[:, mid] = (np.arange(128) < tail).astype(np.float32)

    ident = np.eye(128, dtype=BF16)
    in_maps = []
    for c in range(NCORES):
        k_hi = kb[:, c, :].astype(BF16)
        v_hi = vb[:, c, :].astype(BF16)
        qc = (q.reshape(B, HKV, G, D)[order, c] * SCALE).reshape(B * G, D)
        qT = np.ascontiguousarray(qc.T)
        q_hi = qT.astype(BF16)
        q_lo = (qT - q_hi.astype(np.float32)).astype(BF16)
        im = {
            "kv_il": np.ascontiguousarray(np.concatenate([k_hi, v_hi], axis=1)),
            "qhiT": np.ascontiguousarray(q_hi),
            "qloT": np.ascontiguousarray(q_lo),
            "identd": ident,
            "maskc": maskc,
            "idx_all": idx_all,
        }
        in_maps.append(im)
    return nc, info, in_maps, order


def postprocess(results, info, order, cores=None):
    OC = RPG * G
    out = np.zeros((B, HQ, D), np.float32)
    for c in (cores if cores is not None else range(NCORES)):
        o_un = results[c]["o_un"]  # [128 d, B*G] o^T columns
        sums = results[c]["sums"]
        for g in range(GROUPS):
            gi = info[g]
            for j in range(RPG):
                b = int(order[RPG * g + j])
                stot = np.zeros(G, np.float64)
                for (s0, cnt) in gi["req_ranges"][j]:
                    seg = sums[g, 4 * s0:4 * (s0 + cnt)].astype(np.float64)
                    stot += seg.reshape(cnt, G).sum(axis=0)
                ov = o_un[:, OC * g + G * j:OC * g + G * (j + 1)]  # [128 d, G]
                with np.errstate(divide="ignore", invalid="ignore"):
                    out[b, c * G:(c + 1) * G, :] = (ov / stot[None, :]).T
    return out.reshape(B, HQ * D).astype(np.float32)


def kernel(**inputs):
    global LAST_RESULT
    nc, info, in_maps, order = prepare(inputs)
    res = run_bass_kernel_spmd(nc, in_maps, core_ids=list(range(NCORES)),
                               trace=False)
    LAST_RESULT = res
    return postprocess(res.results, info, order)
